# revision 1
# baseline (speedup 1.0000x reference)
"""Trainium2 Bass kernel for additive (Bahdanau) attention GNN message passing.

score[n, m] = v . tanh(x1[n] @ W1.T + (x2[m] @ W2.T + bc))      (per attendee set)
w = softmax(score, axis=n);  ctx[m] = w[:, m].T @ x1
out = tanh(concat([att, ctx_s, ctx_e]) @ W_lin.T + b_lin)

Sharding: attender dim M=1024 split across 8 cores (128 each); attendees and
params replicated. No collectives — embarrassingly parallel SPMD.

Host-side prep (kernel() wrapper): inputs are re-laid-out (transposes /
bf16 casts / SBUF-image packing — zero FLOPs) so the device never runs
layout plumbing. The attendee image carries a ones-column per chunk so the
softmax sums fall out of the ctx matmuls for free.

Engine split per core: DVE applies the per-attender bias (tensor_scalar_add,
bf16 4x mode), ACT runs one fused tanh over a group of attenders per
instruction (the throughput bottleneck: 25.2M tanh elems / 128 lanes /
1.2GHz ~ 164us), PE contracts tanh tiles against v (t-chunk stationary, v
moving) writing the score matrix TRANSPOSED [n, m] so softmax sums/ctx need
no cross-partition moves. The first two attenders run tanh straight from
the aT PSUM with the per-partition bias operand, hiding the pre-add
pipeline warmup. The softmax/ctx/final-linear/output-DMA epilogue runs per
m-quarter, interleaved into the main loop so only the last quarter trails
the tanh stream.
"""

import numpy as np
from ml_dtypes import bfloat16

import concourse.bass as bass
import concourse.tile as tile
from concourse import bacc, masks, mybir
from concourse.bass_utils import run_bass_kernel_spmd

F32 = mybir.dt.float32
BF16 = mybir.dt.bfloat16
AF = mybir.ActivationFunctionType

H = 128      # hidden
A = 256      # attention (output) size
N_S = 1024   # attendee statements
N_E = 512    # attendee EREs
M = 1024     # attenders
NC = 8       # cores
ML = M // NC # attenders per core
NT = N_S + N_E  # 1536
NCH = NT // 128  # 12 chunks of attendees
CW = 129     # x-image chunk width: 128 attendee cols + a ones column

# fused-tanh group schedule (after the 2 bias-path attenders): ramp-up so
# ACT starts sooner, ramp-down so the last score matmuls overlap the
# preceding tanh
N_BIAS = 2
GROUPS = [2, 4] + [8] * 14 + [4, 2, 2]
assert N_BIAS + sum(GROUPS) == ML
GMAX = max(GROUPS)

_CACHE = {}


def _build():
    nc = bacc.Bacc(
        "TRN2", target_bir_lowering=False, debug=False, num_devices=NC
    )

    d_x16 = nc.dram_tensor("x16", [128, NCH * CW], BF16, kind="ExternalInput").ap()
    d_stmtsT = nc.dram_tensor("stmtsT", [128, N_S], F32, kind="ExternalInput").ap()
    d_eresT = nc.dram_tensor("eresT", [128, N_E], F32, kind="ExternalInput").ap()
    d_attT = nc.dram_tensor("attT", [128, ML], F32, kind="ExternalInput").ap()
    d_wT = nc.dram_tensor("wT", [128, 4 * H], F32, kind="ExternalInput").ap()
    d_wlinT = nc.dram_tensor("wlinT", [128, 3 * A], F32, kind="ExternalInput").ap()
    d_vb = nc.dram_tensor("vb", [128, 4], F32, kind="ExternalInput").ap()
    d_v16 = nc.dram_tensor("v16", [128, 2], BF16, kind="ExternalInput").ap()
    d_blin = nc.dram_tensor("blin", [1, A], F32, kind="ExternalInput").ap()
    d_out = nc.dram_tensor("out", [ML, A], F32, kind="ExternalOutput").ap()

    with tile.TileContext(nc) as tc:
        _emit(nc, tc, d_x16, d_stmtsT, d_eresT, d_attT, d_wT,
              d_wlinT, d_vb, d_v16, d_blin, d_out)

    nc.compile()
    return nc


def _emit(nc, tc, d_x16, d_stmtsT, d_eresT, d_attT, d_wT,
          d_wlinT, d_vb, d_v16, d_blin, d_out):
    from contextlib import ExitStack

    ctx = ExitStack()
    with ctx:
        const = ctx.enter_context(tc.tile_pool(name="const", bufs=1))
        work = ctx.enter_context(tc.tile_pool(name="work", bufs=1))
        tin_pool = ctx.enter_context(tc.tile_pool(name="tin", bufs=3))
        tout_pool = ctx.enter_context(tc.tile_pool(name="tout", bufs=3))
        ps_big = ctx.enter_context(
            tc.tile_pool(name="ps_big", bufs=3, space=bass.MemorySpace.PSUM))
        ps_small = ctx.enter_context(
            tc.tile_pool(name="ps_small", bufs=1, space=bass.MemorySpace.PSUM))
        ps_score = ctx.enter_context(
            tc.tile_pool(name="ps_score", bufs=1, space=bass.MemorySpace.PSUM))

        # ---- gpsimd init ----
        # ident4: four stacked 32x32 identities — rhs for the per-quarter ctx
        # transposes, whose lhsT sits at partition base 32q (operand bases
        # must match)
        ident4 = const.tile([128, 32], F32)
        for q in range(4):
            masks.make_identity(nc, ident4[q * 32:(q + 1) * 32, :])
        scratch = const.tile([128, 1], F32)
        nc.gpsimd.memset(scratch[:], 0.0)
        ones_row = const.tile([1, 128], F32)
        nc.gpsimd.memset(ones_row[:], 1.0)
        # warm the ACT table set (exp_and_others holds tanh+exp) during DMA
        nc.scalar.activation(scratch[:], scratch[:], AF.Tanh)

        # ---- DMA inputs, interleaved with their consumers so each compute
        # op's DMA-queue wait only covers the transfers it actually needs ----
        sb_stmtsT = const.tile([128, N_S], F32)     # [h, n]
        nc.sync.dma_start(sb_stmtsT[:, 0:512], d_stmtsT[:, 0:512])
        sb_wT = const.tile([128, 4 * H], F32)
        nc.sync.dma_start(sb_wT[:], d_wT[:, :])
        sb_attT = const.tile([128, ML], F32)
        nc.gpsimd.dma_start(sb_attT[:], d_attT[:, :])
        sb_vb = const.tile([128, 4], F32)
        nc.gpsimd.dma_start(sb_vb[:], d_vb[:, :])

        sb_v16 = const.tile([128, 2], BF16)
        nc.gpsimd.dma_start(sb_v16[:], d_v16[:, :])

        # aT_s half 0 + bT path
        ps_aTs = [ps_big.tile([128, 512], F32, tag="ps", name=f"ps_aTs{j}")
                  for j in range(2)]
        nc.tensor.matmul(ps_aTs[0][:], sb_wT[:, 0:128], sb_stmtsT[:, 0:512],
                         start=True, stop=True)
        sb_bTs = const.tile([128, ML], F32)
        ps = ps_big.tile([128, 512], F32, tag="ps")
        nc.tensor.matmul(ps[:, 0:ML], sb_wT[:, 128:256], sb_attT[:],
                         start=True, stop=True)
        nc.vector.tensor_scalar_add(sb_bTs[:], ps[:, 0:ML], sb_vb[:, 2:3])
        sb_bTe = const.tile([128, ML], F32)
        ps = ps_big.tile([128, 512], F32, tag="ps")
        nc.tensor.matmul(ps[:, 0:ML], sb_wT[:, 384:512], sb_attT[:],
                         start=True, stop=True)
        nc.vector.tensor_scalar_add(sb_bTe[:], ps[:, 0:ML], sb_vb[:, 3:4])

        nc.gpsimd.dma_start(sb_stmtsT[:, 512:1024], d_stmtsT[:, 512:1024])
        nc.tensor.matmul(ps_aTs[1][:], sb_wT[:, 0:128], sb_stmtsT[:, 512:1024],
                         start=True, stop=True)

        sb_eresT = const.tile([128, N_E], F32)      # [h, n]
        nc.gpsimd.dma_start(sb_eresT[:], d_eresT[:, :])
        ps_aTe = ps_big.tile([128, 512], F32, tag="ps")
        nc.tensor.matmul(ps_aTe[:], sb_wT[:, 256:384], sb_eresT[:],
                         start=True, stop=True)

        sb_blin = const.tile([1, A], F32)
        nc.gpsimd.dma_start(sb_blin[0:1, :], d_blin[0:1, :])

        # epilogue-only tensors: separate (gpsimd) queue
        sb_x16 = const.tile([128, NCH * CW], BF16)  # raw chunks + ones col
        nc.gpsimd.dma_start(sb_x16[:], d_x16[:, :])
        sb_wlinT = const.tile([128, 3 * A], F32)
        nc.gpsimd.dma_start(sb_wlinT[:], d_wlinT[:, :])

        # att + b_lin parts of the final linear: no ctx dependence, so they
        # run during setup, off the epilogue critical path
        ps_out = ps_small.tile([128, A], F32, tag="out")
        for q4 in range(4):
            lo4 = 32 * q4
            nc.tensor.matmul(ps_out[lo4:lo4 + 32, :], sb_attT[:, lo4:lo4 + 32],
                             sb_wlinT[:, 0:A], start=True, stop=False,
                             tile_position=(0, lo4), skip_group_check=True)
            nc.tensor.matmul(ps_out[lo4:lo4 + 32, :], ones_row[0:1, lo4:lo4 + 32],
                             sb_blin[0:1, :], start=False, stop=False,
                             tile_position=(0, lo4), skip_group_check=True)

        # ---- main pipeline state ----
        ps_sT = ps_score.tile([128, NT], F32)    # [n_local, c*128 + m]
        ps_sT3 = ps_sT[:].rearrange("p (c m) -> p c m", c=NCH)
        sb_E = work.tile([128, NT], BF16)        # exp(scoresT), same layout
        sb_E3 = sb_E[:].rearrange("p (c m) -> p c m", c=NCH)
        ps_ctx = ps_small.tile([128, 2 * CW], F32, tag="ctx")  # ctx|sum, s then e
        sb_recip = work.tile([128, 2], F32)
        sb_ctx = work.tile([128, 2 * H], F32)
        sb_ctxT = work.tile([128, 2 * H], F32)
        sb_out = work.tile([128, A], F32)

        def score_mms(t_out, g, m):
            for c in range(NCH):
                v = sb_v16[:, 0:1] if c < 8 else sb_v16[:, 1:2]
                nc.tensor.matmul(ps_sT[:, c * 128 + m: c * 128 + m + 1],
                                 t_out[:, g * NT + c * 128: g * NT + (c + 1) * 128],
                                 v, start=True, stop=True)

        # first two attenders: tanh straight from the aT PSUM tiles with the
        # per-partition bias operand (hides the DVE pre-add warmup)
        t_out0 = tout_pool.tile([128, GMAX * NT], BF16, tag="tout")
        for g in range(N_BIAS):
            m = g
            for j in range(2):
                nc.scalar.activation(t_out0[:, g * NT + j * 512: g * NT + (j + 1) * 512],
                                     ps_aTs[j][:], AF.Tanh, bias=sb_bTs[:, m:m + 1])
            nc.scalar.activation(t_out0[:, g * NT + N_S: (g + 1) * NT],
                                 ps_aTe[:], AF.Tanh, bias=sb_bTe[:, m:m + 1])
        for g in range(N_BIAS):
            score_mms(t_out0, g, g)

        sb_aTs = const.tile([128, N_S], BF16)
        for j in range(2):
            nc.vector.tensor_copy(sb_aTs[:, j * 512:(j + 1) * 512], ps_aTs[j][:])
        sb_aTe = const.tile([128, N_E], BF16)
        nc.vector.tensor_copy(sb_aTe[:], ps_aTe[:])

        def quarter_epilogue(q):
            """softmax + ctx + final linear + out-DMA for m in [32q, 32q+32)."""
            lo = 32 * q
            # E = exp(scoresT) on the quarter's columns of every chunk
            nc.scalar.activation(sb_E3[:, :, lo:lo + 32],
                                 ps_sT3[:, :, lo:lo + 32], AF.Exp)
            # ctx (and, via the ones column, the softmax sum) per chunk
            for c in range(8):
                nc.tensor.matmul(ps_ctx[lo:lo + 32, 0:CW],
                                 sb_E[:, c * 128 + lo: c * 128 + lo + 32],
                                 sb_x16[:, c * CW:(c + 1) * CW],
                                 start=(c == 0), stop=(c == 7),
                                 tile_position=(0, lo))
            for c in range(8, 12):
                nc.tensor.matmul(ps_ctx[lo:lo + 32, CW:2 * CW],
                                 sb_E[:, c * 128 + lo: c * 128 + lo + 32],
                                 sb_x16[:, c * CW:(c + 1) * CW],
                                 start=(c == 8), stop=(c == 11),
                                 tile_position=(0, lo))
            nc.vector.reciprocal(sb_recip[lo:lo + 32, 0:1],
                                 ps_ctx[lo:lo + 32, H:H + 1])
            nc.vector.reciprocal(sb_recip[lo:lo + 32, 1:2],
                                 ps_ctx[lo:lo + 32, CW + H:CW + H + 1])
            nc.vector.tensor_scalar_mul(sb_ctx[lo:lo + 32, 0:H],
                                        ps_ctx[lo:lo + 32, 0:H],
                                        sb_recip[lo:lo + 32, 0:1])
            nc.vector.tensor_scalar_mul(sb_ctx[lo:lo + 32, H:2 * H],
                                        ps_ctx[lo:lo + 32, CW:CW + H],
                                        sb_recip[lo:lo + 32, 1:2])
            # transpose the quarter's ctx rows -> ctxT columns [h, m] (bf16)
            for half in range(2):
                ps_t = ps_big.tile([128, 512], F32, tag="ps")
                nc.tensor.matmul(ps_t[:, 0:32],
                                 sb_ctx[lo:lo + 32, half * H:(half + 1) * H],
                                 ident4[lo:lo + 32, :], is_transpose=True,
                                 tile_position=(lo, 0))
                nc.vector.tensor_copy(sb_ctxT[:, half * H + lo: half * H + lo + 32],
                                      ps_t[:, 0:32])
            # finish the final linear (att+bias parts ran during setup)
            nc.tensor.matmul(ps_out[lo:lo + 32, :], sb_ctxT[:, lo:lo + 32],
                             sb_wlinT[:, A:2 * A], start=False, stop=False,
                             tile_position=(0, lo), skip_group_check=True)
            nc.tensor.matmul(ps_out[lo:lo + 32, :], sb_ctxT[:, H + lo:H + lo + 32],
                             sb_wlinT[:, 2 * A:3 * A], start=False, stop=True,
                             tile_position=(0, lo), skip_group_check=True)
            if q == 3:  # all quarters' final matmuls done -> one tanh + store
                nc.scalar.activation(sb_out[:], ps_out[:], AF.Tanh)
                nc.sync.dma_start(d_out[:, :], sb_out[:])

        # ---- fused main loop ----
        # DVE: t_in[:, g*NT + 0:N_S]   = aT_s + bT_s[:, m]   (bf16 4x mode)
        #      t_in[:, g*NT + N_S:NT]  = aT_e + bT_e[:, m]
        # ACT: t_out = tanh(t_in)  — one [128, grp*NT] instruction
        # PE : scoresT[:, c*128 + m] = t_chunk.T @ v
        m0 = N_BIAS
        next_q = 0
        for grp in GROUPS:
            t_in = tin_pool.tile([128, GMAX * NT], BF16, tag="tin")
            for g in range(grp):
                m = m0 + g
                nc.vector.tensor_scalar_add(t_in[:, g * NT: g * NT + N_S],
                                            sb_aTs[:], sb_bTs[:, m:m + 1])
                nc.vector.tensor_scalar_add(t_in[:, g * NT + N_S: (g + 1) * NT],
                                            sb_aTe[:], sb_bTe[:, m:m + 1])
            t_out = tout_pool.tile([128, GMAX * NT], BF16, tag="tout")
            nc.scalar.activation(t_out[:, 0:grp * NT], t_in[:, 0:grp * NT], AF.Tanh)
            for g in range(grp):
                score_mms(t_out, g, m0 + g)
            m0 += grp
            while next_q < 4 and m0 >= 32 * (next_q + 1):
                quarter_epilogue(next_q)
                next_q += 1


def _get_nc():
    if "nc" not in _CACHE:
        _CACHE["nc"] = _build()
    return _CACHE["nc"]


def _prep_inputs(inputs):
    """Host-side layout prep: transposes / bf16 casts / SBUF-image packing."""
    f = {k: np.ascontiguousarray(np.asarray(v, np.float32))
         for k, v in inputs.items()}
    stmts, eres = f["attendee_stmts"], f["attendee_eres"]
    ws, we, wlin = f["Ws_concat"], f["We_concat"], f["W_lin"]

    # x image: chunk c holds attendees [c*128, (c+1)*128) as [n_local, h],
    # plus a trailing ones column (turns the ctx matmul into ctx|sum)
    x = np.empty((128, NCH * CW), np.float32)
    for c in range(8):
        x[:, c * CW:c * CW + H] = stmts[c * 128:(c + 1) * 128]
        x[:, c * CW + H] = 1.0
    for c in range(8, 12):
        x[:, c * CW:c * CW + H] = eres[(c - 8) * 128:(c - 7) * 128]
        x[:, c * CW + H] = 1.0
    vb = np.ascontiguousarray(
        np.stack([f["vs_single"], f["ve_single"], f["bs_concat"],
                  f["be_concat"]], axis=1))
    shared = {
        "x16": np.ascontiguousarray(x.astype(bfloat16)),
        "stmtsT": np.ascontiguousarray(stmts.T),
        "eresT": np.ascontiguousarray(eres.T),
        "wT": np.ascontiguousarray(np.concatenate(
            [ws[:, :H].T, ws[:, H:].T, we[:, :H].T, we[:, H:].T], axis=1)),
        "wlinT": np.ascontiguousarray(np.concatenate(
            [wlin[:, 0:H].T, wlin[:, H:2 * H].T, wlin[:, 2 * H:3 * H].T], axis=1)),
        "vb": vb,
        "v16": np.ascontiguousarray(vb[:, 0:2].astype(bfloat16)),
        "blin": np.ascontiguousarray(f["b_lin"][None, :]),
    }
    att = f["attender"]
    in_maps = []
    for i in range(NC):
        attT = np.ascontiguousarray(att[i * ML:(i + 1) * ML].T)
        in_maps.append(dict(shared, attT=attT))
    return in_maps


def kernel(**inputs) -> np.ndarray:
    nc = _get_nc()
    in_maps = _prep_inputs(inputs)
    res = run_bass_kernel_spmd(nc, in_maps, list(range(NC)))
    return np.concatenate([res.results[i]["out"] for i in range(NC)], axis=0)

